# revision 1
# baseline (speedup 1.0000x reference)
"""EnhancedDNCMemory forward step on 8 Trainium2 NeuronCores.

Strategy
--------
The only heavy tensor is the temporal link matrix ``link`` [B=4, N=4096,
N=4096] (256 MiB f32). Everything else is O(N) or O(N*W) and is computed on
the host in float32.

The reference computes (per batch)::

    link_new = (1 - w_i - w_j) * link + w_i * p_j          (diag zeroed)
    fwd[r]   = link_new   @ rwp[r]
    bwd[r]   = link_new^T @ rwp[r]

Expanding link_new, both contractions decompose into matvecs against the
*raw* link matrix with the 8 stacked vectors V = [rwp^T | (w*rwp)^T] (N x 8):

    fwd[r,i] = (1-w_i)*(L@rwp_r)_i - (L@(w*rwp_r))_i + w_i*(p.rwp_r) - w_i*p_i*rwp_r_i
    bwd[r,i] = (1-w_i)*(L^T@rwp_r)_i - (L^T@(w*rwp_r))_i + p_i*(w.rwp_r) - w_i*p_i*rwp_r_i

So the device only computes Y1 = L_slab @ V and Y2 = L_slab^T @ V_slab,
streaming each element of ``link`` from HBM exactly once. Sharding: 8 cores =
4 batches x 2 row-slabs of 2048 rows. Y1 slabs concatenate; Y2 partials sum
(both on host, they are [N, 8] per core).

Device kernel (per core, slab [2048, 4096]):
  - Everything streams in float16 with an exact x4096 (power-of-2) host
    prescale; f32 PSUM accumulation. Host divides the 2^24 scale back out.
    Measured end-to-end relative error ~2e-6 (errors average out across the
    4096-term sums).
  - j-outer loop over 8 column tiles of 512, loaded in 1-MiB quarters.
  - Phase A per tile: 64 PE transposes (transpose-mode) land 128x128 blocks
    in PSUM; DVE/ACT copy them to SBUF. Phase B: 16 Y2 matmuls (lhsT =
    V_slab chunk [128,8], rhs = raw slab) + 16 Y1 matmuls (rhs = transposed
    tiles), PSUM-accumulated. Keeping the two phases separate matters: PE
    transpose-mode <-> matmul-mode switches cost ~a pipeline flush each.

Toolchain notes: walrus on this stack allows at most ONE sync-wait per
instruction — _legalize_waits() drops redundant same-engine waits and hoists
the rest onto same-engine NoOps, and the Tile kernel-tail drain is split
into one Drain per outstanding semaphore.
"""

import os

import numpy as np

B = 4
N = 4096
W = 64
R = 4
NCORES = 8
SLAB = N // 2  # rows per core
EPS = 1e-6

_NC = None
LAST_RESULT = None


# ----------------------------------------------------------------- device ---


def _build_program():
    import concourse.bass as bass
    import concourse.mybir as mybir
    from concourse.tile import TileContext

    F16 = mybir.dt.float16
    F32 = mybir.dt.float32

    class SplitDrainTileContext(TileContext):
        """Split the kernel-tail drain: walrus caps sync-waits per inst at 1."""

        def _drain_and_barrier(self, tick_clock, wait_clock):
            from concourse.vector_clock import ScopedClock, VectorClock

            vec = list(tick_clock.global_clock)
            nz = [i for i, t in enumerate(vec) if t > 0]
            for proc in nz:
                pv = VectorClock(
                    [t if j == proc else 0 for j, t in enumerate(vec)]
                )
                d = self.nc.sync.drain()
                wait_clock.add_sem_waits(d.ins, ScopedClock({None: pv}))
            if not nz:
                d = self.nc.sync.drain()
                wait_clock.add_sem_waits(
                    d.ins, ScopedClock({None: tick_clock.global_clock})
                )
            self.nc.all_engine_barrier()
            assert self.sems is not None
            popped = self.nc._tile_sem_poison_stack.pop()
            assert popped is self._sem_poison
            self.nc.clear_and_free_semaphores(list(self.sems.allocated().values()))
            self.nc.all_engine_barrier()

    nc = bass.Bass()
    lmat = nc.dram_tensor("lmat", [SLAB, N], F16, kind="ExternalInput")
    # consts: [vfull(32*8) | vslab(16*8)]
    consts = nc.dram_tensor("consts", [128, 384], F16, kind="ExternalInput")
    y1t = nc.dram_tensor("y1t", [8, SLAB], F32, kind="ExternalOutput")
    y2t = nc.dram_tensor("y2t", [8, N], F32, kind="ExternalOutput")

    NJT = N // 512  # 8 column tiles
    NIC = SLAB // 128  # 16 row chunks

    with SplitDrainTileContext(nc) as tc:
        with (
            tc.tile_pool(name="cpool", bufs=1) as cpool,
            tc.tile_pool(name="lpool", bufs=3) as lpool,
            tc.tile_pool(name="spool", bufs=3) as spool,
            tc.tile_pool(name="y1pool", bufs=1, space="PSUM") as y1pool,
            tc.tile_pool(name="y2pool", bufs=1, space="PSUM") as y2pool,
            tc.tile_pool(name="tpool", bufs=3, space="PSUM") as tpool,
        ):
            # identity comes from gpsimd (no DMA): the first transposes then
            # gate only on the first link-slab quarter.
            identt = cpool.tile([128, 128], F16, name="identt")
            from concourse.masks import make_identity

            make_identity(nc, identt)
            ident = identt[:, :]

            ct = cpool.tile([128, 384], F16)
            vfull = ct[:, 0:256]  # 32 chunks x 8
            vslab = ct[:, 256:384]  # 16 chunks x 8

            # PE observes the identity (Pool) and consts (DMA) sems once.
            scr = tpool.tile([128, 512], F16, tag="tps", name="touch")
            nc.tensor.transpose(scr[:, 0:128], ident, ident)
            nc.tensor.transpose(scr[0:8, 128:256], vslab[:, 0:8], ident)

            py1 = [
                y1pool.tile([8, 512], F32, tag=f"py1_{s}", name=f"py1_{s}")
                for s in range(4)
            ]

            lv = lmat[:, :].rearrange("(c p) (t j) -> p c t j", p=128, j=512)

            ncopy = 0
            for jt in range(NJT):
                slab = lpool.tile([128, NIC, 512], F16, tag="slab")
                # quarter-granularity loads so PE can start after 1 MiB
                for q in range(4):
                    nc.sync.dma_start(
                        slab[:, 4 * q : 4 * q + 4, :], lv[:, 4 * q : 4 * q + 4, jt, :]
                    )
                    if jt == 0 and q == 0:
                        # V constants ride the ring behind the very first
                        # quarter; nothing on PE needs them until phase B.
                        nc.sync.dma_start(ct, consts[:, :])

                # Phase A: all 64 transposes of this column tile, in one
                # uninterrupted transpose-mode run (mode switches between
                # transpose and matmul cost ~a pipeline flush each). The
                # copy engines drain each 4-transpose pack to SBUF.
                sbts = {}
                for pack in range(4):
                    ics = range(4 * pack, 4 * pack + 4)
                    for jcl in range(4):
                        tps = tpool.tile([128, 512], F16, tag="tps", name="tps")
                        for qq, ic in enumerate(ics):
                            nc.tensor.transpose(
                                tps[:, qq * 128 : (qq + 1) * 128],
                                slab[:, ic, jcl * 128 : (jcl + 1) * 128],
                                ident,
                            )
                        sbt = spool.tile(
                            [128, 512], F16, tag="sbt", bufs=20, name="sbt"
                        )
                        if ncopy % 3 != 2:
                            nc.vector.tensor_copy(sbt, tps)
                        else:
                            nc.scalar.copy(sbt, tps)
                        ncopy += 1
                        sbts[(pack, jcl)] = sbt

                # Phase B: one uninterrupted matmul-mode run. Y1 of the first
                # two packs first (their copies completed mid-phase-A), Y2
                # next (raw-slab rhs, giving the remaining copies slack),
                # then Y1 of the last two packs.
                py2 = y2pool.tile([8, 512], F32, tag="py2", name="py2")

                def emit_y1(pack):
                    for jcl in range(4):
                        jc = jt * 4 + jcl
                        nc.tensor.matmul(
                            py1[pack],
                            vfull[:, jc * 8 : (jc + 1) * 8],
                            sbts[(pack, jcl)],
                            start=(jt == 0 and jcl == 0),
                            stop=(jt == NJT - 1 and jcl == 3),
                        )

                emit_y1(0)
                emit_y1(1)
                for ic in range(NIC):
                    nc.tensor.matmul(
                        py2,
                        vslab[:, ic * 8 : (ic + 1) * 8],
                        slab[:, ic, :],
                        start=(ic == 0),
                        stop=(ic == NIC - 1),
                    )
                emit_y1(2)
                emit_y1(3)
                for pack in range(4):
                    if jt == NJT - 1:
                        # py1[pack] just received its stop — drain it now so
                        # the tail shrinks to the last pack only.
                        y1seg = spool.tile(
                            [8, 512], F32, tag="y1s", bufs=4, name="y1seg"
                        )
                        nc.vector.tensor_copy(y1seg, py1[pack])
                        nc.scalar.dma_start(
                            y1t[:, pack * 512 : (pack + 1) * 512], y1seg
                        )

                y2s = spool.tile([8, 512], F32, tag="y2s", bufs=NJT, name="y2s")
                nc.vector.tensor_copy(y2s, py2)
                nc.scalar.dma_start(y2t[:, jt * 512 : (jt + 1) * 512], y2s)
    return nc


def _legalize_waits(nc):
    """Walrus on this toolchain allows at most ONE sync-wait per instruction.

    Two rewrites, applied to the finished BIR:
      1. Drop same-engine waits — every engine queue executes (and completes
         compute instructions) in order, so a wait on the engine's own
         semaphore from within its own stream is implied by program order.
      2. If an instruction still carries more than one wait, hoist all but
         the last onto fresh same-engine InstNoOps inserted just before it.
    """
    import concourse.mybir as mybir

    eng_prefix = {
        mybir.EngineType.PE: "PE_",
        mybir.EngineType.DVE: "DVE_",
        mybir.EngineType.Activation: "ACT_",
        mybir.EngineType.Pool: "Pool_",
        mybir.EngineType.SP: "SP_",
    }
    uid = 0
    for f in nc.m.functions:
        for b in f.blocks:
            out = []
            for inst in b.instructions:
                si = getattr(inst, "sync_info", None)
                waits = list(si.on_wait) if si is not None and si.on_wait else []
                if len(waits) > 1:
                    pref = eng_prefix.get(inst.engine)
                    if pref is not None:
                        keep = [
                            w
                            for w in waits
                            if not (w.ant_name or "").startswith(pref)
                        ]
                        waits = keep if keep else waits[-1:]
                    for w in waits[:-1]:
                        uid += 1
                        out.append(
                            mybir.InstNoOp(
                                name=f"lw-nop-{uid}",
                                engine=inst.engine,
                                sync_info=mybir.SyncInfo(
                                    on_wait=[w], on_update=[]
                                ),
                                bass_nofuse=True,
                            )
                        )
                    inst.sync_info = mybir.SyncInfo(
                        on_wait=waits[-1:],
                        on_update=list(si.on_update or []),
                    )
                out.append(inst)
            b.instructions[:] = out


def _get_nc():
    global _NC
    if _NC is None:
        nc = _build_program()
        _legalize_waits(nc)
        _NC = nc
    return _NC


# ------------------------------------------------------------- host math ---


def _sigmoid(x):
    x = np.asarray(x, np.float32)
    out = np.empty_like(x)
    pos = x >= 0
    out[pos] = 1.0 / (1.0 + np.exp(-x[pos]))
    ex = np.exp(x[~pos])
    out[~pos] = ex / (1.0 + ex)
    return out


def _softplus(x):
    x = np.asarray(x, np.float32)
    return np.log1p(np.exp(-np.abs(x))) + np.maximum(x, 0.0)


def _softmax(x, axis=-1):
    x = np.asarray(x, np.float32)
    m = np.max(x, axis=axis, keepdims=True)
    e = np.exp(x - m)
    return e / np.sum(e, axis=axis, keepdims=True)


def _content_weights(mem, keys, beta):
    # mem: [B,N,W], keys: [B,K,W], beta: [B,K] -> [B,K,N]
    dot = np.einsum("bnw,bkw->bkn", mem, keys, dtype=np.float32)
    mem_n = np.linalg.norm(mem, axis=-1)[:, None, :].astype(np.float32)
    key_n = np.linalg.norm(keys, axis=-1)[:, :, None].astype(np.float32)
    sim = dot / (mem_n * key_n + EPS)
    return _softmax(beta[..., None] * sim, axis=-1)


def _allocation(usage):
    idx = np.argsort(usage, axis=-1, kind="stable")
    sorted_u = np.take_along_axis(usage, idx, axis=-1)
    cp = np.cumprod(sorted_u, axis=-1)
    excl = np.concatenate([np.ones_like(cp[:, :1]), cp[:, :-1]], axis=-1)
    alloc_sorted = ((1.0 - sorted_u) * excl).astype(np.float32)
    out = np.empty_like(alloc_sorted)
    np.put_along_axis(out, idx, alloc_sorted, axis=-1)
    return out


# ----------------------------------------------------------------- kernel ---


def kernel(
    memory,
    usage,
    link,
    precedence,
    read_w_prev,
    write_w_prev,
    write_key,
    write_strength_raw,
    erase_raw,
    write_vec,
    free_raw,
    alloc_gate_raw,
    write_gate_raw,
    read_keys,
    read_strengths_raw,
    read_modes_raw,
):
    global LAST_RESULT
    from concourse.bass_utils import run_bass_kernel_spmd

    f32 = np.float32
    memory = np.asarray(memory, f32)
    usage = np.asarray(usage, f32)
    link = np.asarray(link, f32)
    precedence = np.asarray(precedence, f32)
    read_w_prev = np.asarray(read_w_prev, f32)
    write_w_prev = np.asarray(write_w_prev, f32)
    write_key = np.asarray(write_key, f32)
    write_strength_raw = np.asarray(write_strength_raw, f32)
    erase_raw = np.asarray(erase_raw, f32)
    write_vec = np.asarray(write_vec, f32)
    free_raw = np.asarray(free_raw, f32)
    alloc_gate_raw = np.asarray(alloc_gate_raw, f32)
    write_gate_raw = np.asarray(write_gate_raw, f32)
    read_keys = np.asarray(read_keys, f32)
    read_strengths_raw = np.asarray(read_strengths_raw, f32)
    read_modes_raw = np.asarray(read_modes_raw, f32)

    # --- interface activations ---
    write_strength = 1.0 + _softplus(write_strength_raw)  # [B]
    read_strengths = 1.0 + _softplus(read_strengths_raw)  # [B,R]
    erase = _sigmoid(erase_raw)  # [B,W]
    free = _sigmoid(free_raw)  # [B,R]
    g_a = _sigmoid(alloc_gate_raw)[:, None]  # [B,1]
    g_w = _sigmoid(write_gate_raw)[:, None]  # [B,1]
    modes = _softmax(read_modes_raw, axis=-1)  # [B,R,3]

    # --- write content addressing ---
    c_w = _content_weights(memory, write_key[:, None, :], write_strength[:, None])[
        :, 0
    ]  # [B,N]

    # --- usage update + allocation ---
    retention = np.prod(
        1.0 - free[..., None] * read_w_prev, axis=1, dtype=f32
    )  # [B,N]
    usage_new = ((usage + write_w_prev - usage * write_w_prev) * retention).astype(f32)
    alloc = _allocation(usage_new)  # [B,N]

    # --- write weights, memory erase/write ---
    w_w = (g_w * (g_a * alloc + (1.0 - g_a) * c_w)).astype(f32)  # [B,N]
    memory_new = (
        memory * (1.0 - w_w[:, :, None] * erase[:, None, :])
        + w_w[:, :, None] * write_vec[:, None, :]
    ).astype(f32)  # [B,N,W]

    # --- device part: Y1 = L @ V, Y2 = L^T @ V (per batch, split in 2 slabs) ---
    # V = [rwp^T | (w*rwp)^T]  ->  [N, 8]
    V = np.concatenate(
        [
            read_w_prev.transpose(0, 2, 1),  # [B,N,R]
            (w_w[:, :, None] * read_w_prev.transpose(0, 2, 1)),
        ],
        axis=2,
    ).astype(f32)  # [B,N,8]

    # Device runs fp16 with an exact power-of-2 prescale: values of link and V
    # are O(1/N), so x4096 recenters them into fp16's well-conditioned range.
    # The output scale (4096^2 = 2^24) divides out exactly.
    SCALE = 4096.0
    f16 = np.float16
    V16 = (V * SCALE).astype(f16)
    link16 = (link * SCALE).astype(f16)
    in_maps = []
    for core in range(NCORES):
        b, h = divmod(core, 2)
        r0 = h * SLAB
        vfull = np.ascontiguousarray(
            V16[b].reshape(N // 128, 128, 8).transpose(1, 0, 2).reshape(128, -1)
        )
        vslab = np.ascontiguousarray(
            V16[b, r0 : r0 + SLAB]
            .reshape(SLAB // 128, 128, 8)
            .transpose(1, 0, 2)
            .reshape(128, -1)
        )
        consts = np.concatenate([vfull, vslab], axis=1)
        in_maps.append(
            {
                "lmat": np.ascontiguousarray(link16[b, r0 : r0 + SLAB, :]),
                "consts": consts,
            }
        )

    nc = _get_nc()
    res = run_bass_kernel_spmd(
        nc,
        in_maps,
        list(range(NCORES)),
        trace=bool(os.environ.get("DNC_TRACE")),
    )
    LAST_RESULT = res

    UNSCALE = np.float32(1.0 / (SCALE * SCALE))
    Y1 = np.empty((B, N, 8), f32)
    Y2 = np.zeros((B, N, 8), f32)
    for core in range(NCORES):
        b, h = divmod(core, 2)
        r0 = h * SLAB
        Y1[b, r0 : r0 + SLAB] = res.results[core]["y1t"].T * UNSCALE
        Y2[b] += res.results[core]["y2t"].T * UNSCALE

    A = Y1[..., :R].transpose(0, 2, 1)  # [B,R,N] = (L @ rwp_r)_i
    Bm = Y1[..., R:].transpose(0, 2, 1)  # (L @ (w*rwp_r))_i
    C = Y2[..., :R].transpose(0, 2, 1)  # (L^T @ rwp_r)_i
    D = Y2[..., R:].transpose(0, 2, 1)  # (L^T @ (w*rwp_r))_i

    w = w_w[:, None, :]  # [B,1,N]
    p = precedence[:, None, :]  # [B,1,N]
    s = np.einsum("bn,brn->br", precedence, read_w_prev, dtype=f32)[..., None]
    t = np.einsum("bn,brn->br", w_w, read_w_prev, dtype=f32)[..., None]
    diag = (w * p * read_w_prev).astype(f32)  # [B,R,N]

    fwd_w = ((1.0 - w) * A - Bm + w * s - diag).astype(f32)
    bwd_w = ((1.0 - w) * C - D + p * t - diag).astype(f32)

    # --- read content addressing + combine ---
    c_r = _content_weights(memory_new, read_keys, read_strengths)  # [B,R,N]
    read_w = (
        modes[..., 0:1] * bwd_w + modes[..., 1:2] * c_r + modes[..., 2:3] * fwd_w
    ).astype(f32)
    read_vectors = np.einsum("brn,bnw->brw", read_w, memory_new, dtype=f32)
    return read_vectors.astype(f32)



# revision 6
# speedup vs baseline: 1.6401x; 1.6401x over previous
"""EnhancedDNCMemory forward step on 8 Trainium2 NeuronCores.

Strategy
--------
The only heavy tensor is the temporal link matrix ``link`` [B=4, N=4096,
N=4096]. Everything else is O(N) or O(N*W) and is computed on the host in
float32.

The reference computes (per batch)::

    link_new = (1 - w_i - w_j) * link + w_i * p_j          (diag zeroed)
    fwd[r]   = link_new   @ rwp[r]
    bwd[r]   = link_new^T @ rwp[r]

Expanding link_new, both contractions decompose into matvecs against the
*raw* link matrix with the 8 stacked vectors V = [rwp^T | (w*rwp)^T] (N x 8):

    fwd[r,i] = (1-w_i)*(L@rwp_r)_i - (L@(w*rwp_r))_i + w_i*(p.rwp_r) - w_i*p_i*rwp_r_i
    bwd[r,i] = (1-w_i)*(L^T@rwp_r)_i - (L^T@(w*rwp_r))_i + p_i*(w.rwp_r) - w_i*p_i*rwp_r_i

So the device only computes Y1 = L_slab @ V and Y2 = L_slab^T @ V_slab.
Sharding: 8 cores = 4 batches x 2 row-slabs of 2048 rows. Y1 slabs
concatenate; Y2 partials sum (both on host, they are [N, 8] per core).

Device kernel (v2, fp8 + DoubleRow, no on-chip transposes):
  - Everything streams in fp8 e4m3 with an exact x4096 power-of-2 host
    prescale (values are O(1/N)); f32 PSUM accumulation. The 2^24 output
    scale divides out exactly. End-to-end relative error ~2e-4 (quantization
    errors average out across the 4096-term sums).
  - The host uploads TWO fp8 copies of the slab: the native layout (feeds
    Y2 = L^T V, contraction over rows = partitions) and the pre-transposed
    layout (feeds Y1 = L V as (L^T)^T V, again contraction over partitions).
    16.8 MB of DMA per core vs. ~31 us of PE work: the kernel is DMA-bound
    at ~358 GB/s, and the PE never runs a transpose.
  - All matmuls use MatmulPerfMode.DoubleRow: both operands are fp8 tiles
    shaped [128, 2, F] so each PE column-cycle contracts 256 elements --
    2x the bf16 ingestion rate. V chunks ([128, 2, 8]) are the stationary
    operand; the link tiles ([128, 2, 512], free size 1024 = fp8 moving
    max) stream through.
  - Both HBM copies are host-preswizzled so every DMA reads 8 KiB
    contiguous per partition (128 descriptors x 8 KiB per 1 MiB tile).

Toolchain notes: walrus on this stack allows at most ONE sync-wait per
instruction -- _legalize_waits() drops redundant same-engine waits and hoists
the rest onto same-engine NoOps, and the Tile kernel-tail drain is split
into one Drain per outstanding semaphore.
"""

import os

import ml_dtypes
import numpy as np

B = 4
N = 4096
W = 64
R = 4
NCORES = 8
SLAB = N // 2  # rows per core
EPS = 1e-6

NJT = N // 512  # 8 column tiles (Y2)
NIB = SLAB // 512  # 4 output row blocks (Y1)

_NC = None
LAST_RESULT = None


# ----------------------------------------------------------------- device ---


def _build_program():
    import concourse.bass as bass
    import concourse.mybir as mybir
    from concourse.tile import TileContext

    F8 = mybir.dt.float8e4
    F32 = mybir.dt.float32
    DR = mybir.MatmulPerfMode.DoubleRow

    class SplitDrainTileContext(TileContext):
        """Split the kernel-tail drain: walrus caps sync-waits per inst at 1."""

        def _drain_and_barrier(self, tick_clock, wait_clock):
            from concourse.vector_clock import ScopedClock, VectorClock

            vec = list(tick_clock.global_clock)
            nz = [i for i, t in enumerate(vec) if t > 0]
            for proc in nz:
                pv = VectorClock(
                    [t if j == proc else 0 for j, t in enumerate(vec)]
                )
                d = self.nc.sync.drain()
                wait_clock.add_sem_waits(d.ins, ScopedClock({None: pv}))
            if not nz:
                d = self.nc.sync.drain()
                wait_clock.add_sem_waits(
                    d.ins, ScopedClock({None: tick_clock.global_clock})
                )
            self.nc.all_engine_barrier()
            assert self.sems is not None
            popped = self.nc._tile_sem_poison_stack.pop()
            assert popped is self._sem_poison
            self.nc.clear_and_free_semaphores(list(self.sems.allocated().values()))
            self.nc.all_engine_barrier()

    nc = bass.Bass()
    # native slab, preswizzled: [p, t*8192 + k*1024 + m*512 + n]
    #   = L[r0 + 256k + 128m + p, 512t + n]
    lmat = nc.dram_tensor("lmat", [128, NJT * 8192], F8, kind="ExternalInput")
    # transposed slab, preswizzled: [p, v*16384 + k*1024 + m*512 + n]
    #   = L[r0 + 512v + n, 256k + 128m + p]
    tmat = nc.dram_tensor("tmat", [128, NIB * 16384], F8, kind="ExternalInput")
    # V chunks: [p, q*32 + m*16 + c]; q<8: VS[256q+128m+p, c], q>=8: VF[...]
    # (c padded 8 -> 16: dual-fp8 LDWEIGHTS requires the Ko=2 interleave
    # step to be a multiple of 16 bytes -- walrus s3_lw_dual_fp8_restrictions)
    consts = nc.dram_tensor("consts", [128, 768], F8, kind="ExternalInput")
    y1t = nc.dram_tensor("y1t", [8, SLAB], F32, kind="ExternalOutput")
    y2t = nc.dram_tensor("y2t", [8, N], F32, kind="ExternalOutput")

    lv = lmat[:, :].rearrange("p (t k m n) -> p t k m n", t=NJT, k=8, m=2, n=512)
    tv = tmat[:, :].rearrange("p (v k m n) -> p v k m n", v=NIB, k=16, m=2, n=512)
    cv = consts[:, :].rearrange("p (q m c) -> p q m c", m=2, c=16)

    with SplitDrainTileContext(nc) as tc:
        with (
            tc.tile_pool(name="cpool", bufs=1) as cpool,
            tc.tile_pool(name="l2pool", bufs=3) as l2pool,
            tc.tile_pool(name="l1pool", bufs=3) as l1pool,
            tc.tile_pool(name="spool", bufs=3) as spool,
            tc.tile_pool(name="y1pool", bufs=2, space="PSUM") as y1pool,
            tc.tile_pool(name="y2pool", bufs=2, space="PSUM") as y2pool,
        ):
            ct = cpool.tile([128, 24, 2, 16], F8)
            nc.sync.dma_start(ct, cv)

            # --- Y2 = L_slab^T @ V_slab : native tiles, contraction over rows
            for t in range(NJT):
                slab = l2pool.tile([128, 8, 2, 512], F8, tag="l2")
                nc.sync.dma_start(slab, lv[:, t])
                py2 = y2pool.tile([8, 512], F32, tag="py2", name="py2")
                for k in range(8):
                    nc.tensor.matmul(
                        py2,
                        ct[:, k, :, 0:8],
                        slab[:, k],
                        start=(k == 0),
                        stop=(k == 7),
                        perf_mode=DR,
                    )
                y2s = spool.tile([8, 512], F32, tag="y2s", bufs=3, name="y2s")
                nc.vector.tensor_copy(y2s, py2)
                nc.scalar.dma_start(y2t[:, t * 512 : (t + 1) * 512], y2s)

            # --- Y1 = L_slab @ V_full : pre-transposed tiles
            for v in range(NIB):
                tslab = l1pool.tile([128, 16, 2, 512], F8, tag="l1")
                nc.sync.dma_start(tslab, tv[:, v])
                py1 = y1pool.tile([8, 512], F32, tag="py1", name="py1")
                for k in range(16):
                    nc.tensor.matmul(
                        py1,
                        ct[:, 8 + k, :, 0:8],
                        tslab[:, k],
                        start=(k == 0),
                        stop=(k == 15),
                        perf_mode=DR,
                    )
                y1s = spool.tile([8, 512], F32, tag="y1s", bufs=2, name="y1s")
                nc.vector.tensor_copy(y1s, py1)
                nc.scalar.dma_start(y1t[:, v * 512 : (v + 1) * 512], y1s)
    return nc


def _legalize_waits(nc):
    """Walrus on this toolchain allows at most ONE sync-wait per instruction.

    Two rewrites, applied to the finished BIR:
      1. Drop same-engine waits — every engine queue executes (and completes
         compute instructions) in order, so a wait on the engine's own
         semaphore from within its own stream is implied by program order.
      2. If an instruction still carries more than one wait, hoist all but
         the last onto fresh same-engine InstNoOps inserted just before it.
    """
    import concourse.mybir as mybir

    eng_prefix = {
        mybir.EngineType.PE: "PE_",
        mybir.EngineType.DVE: "DVE_",
        mybir.EngineType.Activation: "ACT_",
        mybir.EngineType.Pool: "Pool_",
        mybir.EngineType.SP: "SP_",
    }
    uid = 0
    for f in nc.m.functions:
        for b in f.blocks:
            out = []
            for inst in b.instructions:
                si = getattr(inst, "sync_info", None)
                waits = list(si.on_wait) if si is not None and si.on_wait else []
                if len(waits) > 1:
                    pref = eng_prefix.get(inst.engine)
                    if pref is not None:
                        keep = [
                            w
                            for w in waits
                            if not (w.ant_name or "").startswith(pref)
                        ]
                        waits = keep if keep else waits[-1:]
                    for w in waits[:-1]:
                        uid += 1
                        out.append(
                            mybir.InstNoOp(
                                name=f"lw-nop-{uid}",
                                engine=inst.engine,
                                sync_info=mybir.SyncInfo(
                                    on_wait=[w], on_update=[]
                                ),
                                bass_nofuse=True,
                            )
                        )
                    inst.sync_info = mybir.SyncInfo(
                        on_wait=waits[-1:],
                        on_update=list(si.on_update or []),
                    )
                out.append(inst)
            b.instructions[:] = out


def _get_nc():
    global _NC
    if _NC is None:
        nc = _build_program()
        _legalize_waits(nc)
        _NC = nc
    return _NC


# ------------------------------------------------------------- host math ---


def _sigmoid(x):
    x = np.asarray(x, np.float32)
    out = np.empty_like(x)
    pos = x >= 0
    out[pos] = 1.0 / (1.0 + np.exp(-x[pos]))
    ex = np.exp(x[~pos])
    out[~pos] = ex / (1.0 + ex)
    return out


def _softplus(x):
    x = np.asarray(x, np.float32)
    return np.log1p(np.exp(-np.abs(x))) + np.maximum(x, 0.0)


def _softmax(x, axis=-1):
    x = np.asarray(x, np.float32)
    m = np.max(x, axis=axis, keepdims=True)
    e = np.exp(x - m)
    return e / np.sum(e, axis=axis, keepdims=True)


def _content_weights(mem, keys, beta):
    # mem: [B,N,W], keys: [B,K,W], beta: [B,K] -> [B,K,N]
    dot = np.einsum("bnw,bkw->bkn", mem, keys, dtype=np.float32)
    mem_n = np.linalg.norm(mem, axis=-1)[:, None, :].astype(np.float32)
    key_n = np.linalg.norm(keys, axis=-1)[:, :, None].astype(np.float32)
    sim = dot / (mem_n * key_n + EPS)
    return _softmax(beta[..., None] * sim, axis=-1)


def _allocation(usage):
    idx = np.argsort(usage, axis=-1, kind="stable")
    sorted_u = np.take_along_axis(usage, idx, axis=-1)
    cp = np.cumprod(sorted_u, axis=-1)
    excl = np.concatenate([np.ones_like(cp[:, :1]), cp[:, :-1]], axis=-1)
    alloc_sorted = ((1.0 - sorted_u) * excl).astype(np.float32)
    out = np.empty_like(alloc_sorted)
    np.put_along_axis(out, idx, alloc_sorted, axis=-1)
    return out


# ----------------------------------------------------------------- kernel ---


def kernel(
    memory,
    usage,
    link,
    precedence,
    read_w_prev,
    write_w_prev,
    write_key,
    write_strength_raw,
    erase_raw,
    write_vec,
    free_raw,
    alloc_gate_raw,
    write_gate_raw,
    read_keys,
    read_strengths_raw,
    read_modes_raw,
):
    global LAST_RESULT
    from concourse.bass_utils import run_bass_kernel_spmd

    f32 = np.float32
    memory = np.asarray(memory, f32)
    usage = np.asarray(usage, f32)
    link = np.asarray(link, f32)
    precedence = np.asarray(precedence, f32)
    read_w_prev = np.asarray(read_w_prev, f32)
    write_w_prev = np.asarray(write_w_prev, f32)
    write_key = np.asarray(write_key, f32)
    write_strength_raw = np.asarray(write_strength_raw, f32)
    erase_raw = np.asarray(erase_raw, f32)
    write_vec = np.asarray(write_vec, f32)
    free_raw = np.asarray(free_raw, f32)
    alloc_gate_raw = np.asarray(alloc_gate_raw, f32)
    write_gate_raw = np.asarray(write_gate_raw, f32)
    read_keys = np.asarray(read_keys, f32)
    read_strengths_raw = np.asarray(read_strengths_raw, f32)
    read_modes_raw = np.asarray(read_modes_raw, f32)

    # --- interface activations ---
    write_strength = 1.0 + _softplus(write_strength_raw)  # [B]
    read_strengths = 1.0 + _softplus(read_strengths_raw)  # [B,R]
    erase = _sigmoid(erase_raw)  # [B,W]
    free = _sigmoid(free_raw)  # [B,R]
    g_a = _sigmoid(alloc_gate_raw)[:, None]  # [B,1]
    g_w = _sigmoid(write_gate_raw)[:, None]  # [B,1]
    modes = _softmax(read_modes_raw, axis=-1)  # [B,R,3]

    # --- write content addressing ---
    c_w = _content_weights(memory, write_key[:, None, :], write_strength[:, None])[
        :, 0
    ]  # [B,N]

    # --- usage update + allocation ---
    retention = np.prod(
        1.0 - free[..., None] * read_w_prev, axis=1, dtype=f32
    )  # [B,N]
    usage_new = ((usage + write_w_prev - usage * write_w_prev) * retention).astype(f32)
    alloc = _allocation(usage_new)  # [B,N]

    # --- write weights, memory erase/write ---
    w_w = (g_w * (g_a * alloc + (1.0 - g_a) * c_w)).astype(f32)  # [B,N]
    memory_new = (
        memory * (1.0 - w_w[:, :, None] * erase[:, None, :])
        + w_w[:, :, None] * write_vec[:, None, :]
    ).astype(f32)  # [B,N,W]

    # --- device part: Y1 = L @ V, Y2 = L^T @ V (per batch, split in 2 slabs) ---
    # V = [rwp^T | (w*rwp)^T]  ->  [N, 8]
    V = np.concatenate(
        [
            read_w_prev.transpose(0, 2, 1),  # [B,N,R]
            (w_w[:, :, None] * read_w_prev.transpose(0, 2, 1)),
        ],
        axis=2,
    ).astype(f32)  # [B,N,8]

    # Device runs fp8 e4m3 with an exact power-of-2 prescale: values of link
    # and V are O(1/N), so x4096 recenters them into fp8's normal range. The
    # output scale (4096^2 = 2^24) divides out exactly.
    SCALE = 4096.0
    f8 = ml_dtypes.float8_e4m3
    V8 = np.clip(V * SCALE, -240.0, 240.0).astype(f8)
    link8 = (link * SCALE).astype(f8)

    in_maps = []
    for core in range(NCORES):
        b, h = divmod(core, 2)
        r0 = h * SLAB
        nat = link8[b, r0 : r0 + SLAB, :]  # [2048, 4096]
        # rows r0 + 256k + 128m + p, cols 512t + n -> [p, t, k, m, n]
        lm = np.ascontiguousarray(
            nat.reshape(8, 2, 128, NJT, 512)
            .transpose(2, 3, 0, 1, 4)
            .reshape(128, NJT * 8192)
        )
        tr = nat.T  # [4096, 2048]: [j, i-r0]
        tm = np.ascontiguousarray(
            tr.reshape(16, 2, 128, NIB, 512)
            .transpose(2, 3, 0, 1, 4)
            .reshape(128, NIB * 16384)
        )
        VS = V8[b, r0 : r0 + SLAB]  # [2048, 8]
        VF = V8[b]  # [4096, 8]
        cs = VS.reshape(8, 2, 128, 8).transpose(2, 0, 1, 3)  # [128, 8, 2, 8]
        cf = VF.reshape(16, 2, 128, 8).transpose(2, 0, 1, 3)  # [128, 16, 2, 8]
        cq = np.concatenate([cs, cf], axis=1)  # [128, 24, 2, 8]
        cpad = np.zeros((128, 24, 2, 16), dtype=f8)
        cpad[:, :, :, 0:8] = cq
        consts = np.ascontiguousarray(cpad.reshape(128, 768))
        in_maps.append({"lmat": lm, "tmat": tm, "consts": consts})

    nc = _get_nc()
    res = run_bass_kernel_spmd(
        nc,
        in_maps,
        list(range(NCORES)),
        trace=bool(os.environ.get("DNC_TRACE")),
    )
    LAST_RESULT = res

    UNSCALE = np.float32(1.0 / (SCALE * SCALE))
    Y1 = np.empty((B, N, 8), f32)
    Y2 = np.zeros((B, N, 8), f32)
    for core in range(NCORES):
        b, h = divmod(core, 2)
        r0 = h * SLAB
        Y1[b, r0 : r0 + SLAB] = res.results[core]["y1t"].T * UNSCALE
        Y2[b] += res.results[core]["y2t"].T * UNSCALE

    A = Y1[..., :R].transpose(0, 2, 1)  # [B,R,N] = (L @ rwp_r)_i
    Bm = Y1[..., R:].transpose(0, 2, 1)  # (L @ (w*rwp_r))_i
    C = Y2[..., :R].transpose(0, 2, 1)  # (L^T @ rwp_r)_i
    D = Y2[..., R:].transpose(0, 2, 1)  # (L^T @ (w*rwp_r))_i

    w = w_w[:, None, :]  # [B,1,N]
    p = precedence[:, None, :]  # [B,1,N]
    s = np.einsum("bn,brn->br", precedence, read_w_prev, dtype=f32)[..., None]
    t = np.einsum("bn,brn->br", w_w, read_w_prev, dtype=f32)[..., None]
    diag = (w * p * read_w_prev).astype(f32)  # [B,R,N]

    fwd_w = ((1.0 - w) * A - Bm + w * s - diag).astype(f32)
    bwd_w = ((1.0 - w) * C - D + p * t - diag).astype(f32)

    # --- read content addressing + combine ---
    c_r = _content_weights(memory_new, read_keys, read_strengths)  # [B,R,N]
    read_w = (
        modes[..., 0:1] * bwd_w + modes[..., 1:2] * c_r + modes[..., 2:3] * fwd_w
    ).astype(f32)
    read_vectors = np.einsum("brn,bnw->brw", read_w, memory_new, dtype=f32)
    return read_vectors.astype(f32)


# revision 8
# speedup vs baseline: 1.6489x; 1.0054x over previous
"""EnhancedDNCMemory forward step on 8 Trainium2 NeuronCores.

Strategy
--------
The only heavy tensor is the temporal link matrix ``link`` [B=4, N=4096,
N=4096]. Everything else is O(N) or O(N*W) and is computed on the host in
float32.

The reference computes (per batch)::

    link_new = (1 - w_i - w_j) * link + w_i * p_j          (diag zeroed)
    fwd[r]   = link_new   @ rwp[r]
    bwd[r]   = link_new^T @ rwp[r]

Expanding link_new, both contractions decompose into matvecs against the
*raw* link matrix with the 8 stacked vectors V = [rwp^T | (w*rwp)^T] (N x 8):

    fwd[r,i] = (1-w_i)*(L@rwp_r)_i - (L@(w*rwp_r))_i + w_i*(p.rwp_r) - w_i*p_i*rwp_r_i
    bwd[r,i] = (1-w_i)*(L^T@rwp_r)_i - (L^T@(w*rwp_r))_i + p_i*(w.rwp_r) - w_i*p_i*rwp_r_i

So the device only computes Y1 = L_slab @ V and Y2 = L_slab^T @ V_slab.
Sharding: 8 cores = 4 batches x 2 row-slabs of 2048 rows. Y1 slabs
concatenate; Y2 partials sum (both on host, they are [N, 8] per core).

Device kernel (v2, fp8 + DoubleRow, no on-chip transposes):
  - Everything streams in fp8 e4m3 with an exact x4096 power-of-2 host
    prescale (values are O(1/N)); f32 PSUM accumulation. The 2^24 output
    scale divides out exactly. End-to-end relative error ~2e-4 (quantization
    errors average out across the 4096-term sums).
  - The host uploads TWO fp8 copies of the slab: the native layout (feeds
    Y2 = L^T V, contraction over rows = partitions) and the pre-transposed
    layout (feeds Y1 = L V as (L^T)^T V, again contraction over partitions).
    16.8 MB of DMA per core vs. ~31 us of PE work: the kernel is DMA-bound
    at ~358 GB/s, and the PE never runs a transpose.
  - All matmuls use MatmulPerfMode.DoubleRow: both operands are fp8 tiles
    shaped [128, 2, F] so each PE column-cycle contracts 256 elements --
    2x the bf16 ingestion rate. V chunks ([128, 2, 8]) are the stationary
    operand; the link tiles ([128, 2, 512], free size 1024 = fp8 moving
    max) stream through.
  - Both HBM copies are host-preswizzled so every DMA reads 8 KiB
    contiguous per partition (128 descriptors x 8 KiB per 1 MiB tile).

Toolchain notes: walrus on this stack allows at most ONE sync-wait per
instruction -- _legalize_waits() drops redundant same-engine waits and hoists
the rest onto same-engine NoOps, and the Tile kernel-tail drain is split
into one Drain per outstanding semaphore.
"""

import os

import ml_dtypes
import numpy as np

B = 4
N = 4096
W = 64
R = 4
NCORES = 8
SLAB = N // 2  # rows per core
EPS = 1e-6

NJT = N // 512  # 8 column tiles (Y2)
NIB = SLAB // 512  # 4 output row blocks (Y1)

_NC = None
LAST_RESULT = None


# ----------------------------------------------------------------- device ---


def _build_program():
    import concourse.bass as bass
    import concourse.mybir as mybir
    from concourse.tile import TileContext

    F8 = mybir.dt.float8e4
    F32 = mybir.dt.float32
    DR = mybir.MatmulPerfMode.DoubleRow

    class SplitDrainTileContext(TileContext):
        """Split the kernel-tail drain: walrus caps sync-waits per inst at 1."""

        def _drain_and_barrier(self, tick_clock, wait_clock):
            from concourse.vector_clock import ScopedClock, VectorClock

            vec = list(tick_clock.global_clock)
            nz = [i for i, t in enumerate(vec) if t > 0]
            for proc in nz:
                pv = VectorClock(
                    [t if j == proc else 0 for j, t in enumerate(vec)]
                )
                d = self.nc.sync.drain()
                wait_clock.add_sem_waits(d.ins, ScopedClock({None: pv}))
            if not nz:
                d = self.nc.sync.drain()
                wait_clock.add_sem_waits(
                    d.ins, ScopedClock({None: tick_clock.global_clock})
                )
            self.nc.all_engine_barrier()
            assert self.sems is not None
            popped = self.nc._tile_sem_poison_stack.pop()
            assert popped is self._sem_poison
            self.nc.clear_and_free_semaphores(list(self.sems.allocated().values()))
            self.nc.all_engine_barrier()

    nc = bass.Bass()
    # native slab, preswizzled: [p, t*8192 + k*1024 + m*512 + n]
    #   = L[r0 + 256k + 128m + p, 512t + n]
    lmat = nc.dram_tensor("lmat", [128, NJT * 8192], F8, kind="ExternalInput")
    # transposed slab, preswizzled: [p, v*16384 + k*1024 + m*512 + n]
    #   = L[r0 + 512v + n, 256k + 128m + p]
    tmat = nc.dram_tensor("tmat", [128, NIB * 16384], F8, kind="ExternalInput")
    # V chunks: [p, q*32 + m*16 + c]; q<8: VS[256q+128m+p, c], q>=8: VF[...]
    # (c padded 8 -> 16: dual-fp8 LDWEIGHTS requires the Ko=2 interleave
    # step to be a multiple of 16 bytes -- walrus s3_lw_dual_fp8_restrictions)
    consts = nc.dram_tensor("consts", [128, 768], F8, kind="ExternalInput")
    y1t = nc.dram_tensor("y1t", [8, SLAB], F32, kind="ExternalOutput")
    y2t = nc.dram_tensor("y2t", [8, N], F32, kind="ExternalOutput")

    lv = lmat[:, :].rearrange("p (t k m n) -> p t k m n", t=NJT, k=8, m=2, n=512)
    tv = tmat[:, :].rearrange("p (v k m n) -> p v k m n", v=NIB, k=16, m=2, n=512)
    cv = consts[:, :].rearrange("p (q m c) -> p q m c", m=2, c=16)

    with SplitDrainTileContext(nc) as tc:
        with (
            tc.tile_pool(name="cpool", bufs=1) as cpool,
            tc.tile_pool(name="l2pool", bufs=3) as l2pool,
            tc.tile_pool(name="l1pool", bufs=3) as l1pool,
            tc.tile_pool(name="spool", bufs=3) as spool,
            tc.tile_pool(name="y1pool", bufs=2, space="PSUM") as y1pool,
            tc.tile_pool(name="y2pool", bufs=2, space="PSUM") as y2pool,
        ):
            ct = cpool.tile([128, 24, 2, 16], F8)
            nc.sync.dma_start(ct, cv)

            # Input loads alternate between the two HWDGE rings (SP via
            # nc.sync, ACT via nc.scalar): descriptor generation for a
            # 128-partition transfer costs ~2.6 us per ring, so a single
            # ring can't keep 16 SDMA engines fed during the ramp.
            ring = [nc.sync, nc.scalar]
            nring = 0

            def load(dst, src, nsplit=1):
                nonlocal nring
                ksz = dst.shape[1] // nsplit
                for s in range(nsplit):
                    ring[nring % 2].dma_start(
                        dst[:, s * ksz : (s + 1) * ksz], src[:, s * ksz : (s + 1) * ksz]
                    )
                    nring += 1

            # --- Y1 = L_slab @ V_full : pre-transposed tiles (2 MiB each)
            def do_y1(v, nsplit):
                tslab = l1pool.tile([128, 16, 2, 512], F8, tag="l1")
                load(tslab, tv[:, v], nsplit)
                py1 = y1pool.tile([8, 512], F32, tag="py1", name="py1")
                for k in range(16):
                    nc.tensor.matmul(
                        py1,
                        ct[:, 8 + k, :, 0:8],
                        tslab[:, k],
                        start=(k == 0),
                        stop=(k == 15),
                        perf_mode=DR,
                    )
                y1s = spool.tile([8, 512], F32, tag="y1s", bufs=2, name="y1s")
                nc.vector.tensor_copy(y1s, py1)
                nc.scalar.dma_start(y1t[:, v * 512 : (v + 1) * 512], y1s)

            # --- Y2 = L_slab^T @ V_slab : native tiles (two 512-col tiles
            # per 2 MiB load; the trailing loads stay at 1 MiB to shrink the
            # PE tail behind the last arrival)
            def do_y2(t0, nt):
                slab = l2pool.tile([128, nt, 8, 2, 512], F8, tag=f"l2{nt}")
                load(slab, lv[:, t0 : t0 + nt])
                for t in range(t0, t0 + nt):
                    py2 = y2pool.tile([8, 512], F32, tag="py2", name="py2")
                    for k in range(8):
                        nc.tensor.matmul(
                            py2,
                            ct[:, k, :, 0:8],
                            slab[:, t - t0, k],
                            start=(k == 0),
                            stop=(k == 7),
                            perf_mode=DR,
                        )
                    y2s = spool.tile([8, 512], F32, tag="y2s", bufs=3, name="y2s")
                    nc.vector.tensor_copy(y2s, py2)
                    nc.scalar.dma_start(y2t[:, t * 512 : (t + 1) * 512], y2s)

            do_y1(0, 4)  # first load split 4x for fast DMA ramp
            do_y1(1, 2)
            do_y1(2, 1)
            do_y1(3, 1)
            do_y2(0, 2)
            do_y2(2, 2)
            do_y2(4, 2)
            do_y2(6, 1)
            do_y2(7, 1)
    return nc


def _legalize_waits(nc):
    """Walrus on this toolchain allows at most ONE sync-wait per instruction.

    Two rewrites, applied to the finished BIR:
      1. Drop same-engine waits — every engine queue executes (and completes
         compute instructions) in order, so a wait on the engine's own
         semaphore from within its own stream is implied by program order.
      2. If an instruction still carries more than one wait, hoist all but
         the last onto fresh same-engine InstNoOps inserted just before it.
    """
    import concourse.mybir as mybir

    eng_prefix = {
        mybir.EngineType.PE: "PE_",
        mybir.EngineType.DVE: "DVE_",
        mybir.EngineType.Activation: "ACT_",
        mybir.EngineType.Pool: "Pool_",
        mybir.EngineType.SP: "SP_",
    }
    uid = 0
    for f in nc.m.functions:
        for b in f.blocks:
            out = []
            for inst in b.instructions:
                si = getattr(inst, "sync_info", None)
                waits = list(si.on_wait) if si is not None and si.on_wait else []
                if len(waits) > 1:
                    pref = eng_prefix.get(inst.engine)
                    if pref is not None:
                        keep = [
                            w
                            for w in waits
                            if not (w.ant_name or "").startswith(pref)
                        ]
                        waits = keep if keep else waits[-1:]
                    for w in waits[:-1]:
                        uid += 1
                        out.append(
                            mybir.InstNoOp(
                                name=f"lw-nop-{uid}",
                                engine=inst.engine,
                                sync_info=mybir.SyncInfo(
                                    on_wait=[w], on_update=[]
                                ),
                                bass_nofuse=True,
                            )
                        )
                    inst.sync_info = mybir.SyncInfo(
                        on_wait=waits[-1:],
                        on_update=list(si.on_update or []),
                    )
                out.append(inst)
            b.instructions[:] = out


def _get_nc():
    global _NC
    if _NC is None:
        nc = _build_program()
        _legalize_waits(nc)
        _NC = nc
    return _NC


# ------------------------------------------------------------- host math ---


def _sigmoid(x):
    x = np.asarray(x, np.float32)
    out = np.empty_like(x)
    pos = x >= 0
    out[pos] = 1.0 / (1.0 + np.exp(-x[pos]))
    ex = np.exp(x[~pos])
    out[~pos] = ex / (1.0 + ex)
    return out


def _softplus(x):
    x = np.asarray(x, np.float32)
    return np.log1p(np.exp(-np.abs(x))) + np.maximum(x, 0.0)


def _softmax(x, axis=-1):
    x = np.asarray(x, np.float32)
    m = np.max(x, axis=axis, keepdims=True)
    e = np.exp(x - m)
    return e / np.sum(e, axis=axis, keepdims=True)


def _content_weights(mem, keys, beta):
    # mem: [B,N,W], keys: [B,K,W], beta: [B,K] -> [B,K,N]
    dot = np.einsum("bnw,bkw->bkn", mem, keys, dtype=np.float32)
    mem_n = np.linalg.norm(mem, axis=-1)[:, None, :].astype(np.float32)
    key_n = np.linalg.norm(keys, axis=-1)[:, :, None].astype(np.float32)
    sim = dot / (mem_n * key_n + EPS)
    return _softmax(beta[..., None] * sim, axis=-1)


def _allocation(usage):
    idx = np.argsort(usage, axis=-1, kind="stable")
    sorted_u = np.take_along_axis(usage, idx, axis=-1)
    cp = np.cumprod(sorted_u, axis=-1)
    excl = np.concatenate([np.ones_like(cp[:, :1]), cp[:, :-1]], axis=-1)
    alloc_sorted = ((1.0 - sorted_u) * excl).astype(np.float32)
    out = np.empty_like(alloc_sorted)
    np.put_along_axis(out, idx, alloc_sorted, axis=-1)
    return out


# ----------------------------------------------------------------- kernel ---


def kernel(
    memory,
    usage,
    link,
    precedence,
    read_w_prev,
    write_w_prev,
    write_key,
    write_strength_raw,
    erase_raw,
    write_vec,
    free_raw,
    alloc_gate_raw,
    write_gate_raw,
    read_keys,
    read_strengths_raw,
    read_modes_raw,
):
    global LAST_RESULT
    from concourse.bass_utils import run_bass_kernel_spmd

    f32 = np.float32
    memory = np.asarray(memory, f32)
    usage = np.asarray(usage, f32)
    link = np.asarray(link, f32)
    precedence = np.asarray(precedence, f32)
    read_w_prev = np.asarray(read_w_prev, f32)
    write_w_prev = np.asarray(write_w_prev, f32)
    write_key = np.asarray(write_key, f32)
    write_strength_raw = np.asarray(write_strength_raw, f32)
    erase_raw = np.asarray(erase_raw, f32)
    write_vec = np.asarray(write_vec, f32)
    free_raw = np.asarray(free_raw, f32)
    alloc_gate_raw = np.asarray(alloc_gate_raw, f32)
    write_gate_raw = np.asarray(write_gate_raw, f32)
    read_keys = np.asarray(read_keys, f32)
    read_strengths_raw = np.asarray(read_strengths_raw, f32)
    read_modes_raw = np.asarray(read_modes_raw, f32)

    # --- interface activations ---
    write_strength = 1.0 + _softplus(write_strength_raw)  # [B]
    read_strengths = 1.0 + _softplus(read_strengths_raw)  # [B,R]
    erase = _sigmoid(erase_raw)  # [B,W]
    free = _sigmoid(free_raw)  # [B,R]
    g_a = _sigmoid(alloc_gate_raw)[:, None]  # [B,1]
    g_w = _sigmoid(write_gate_raw)[:, None]  # [B,1]
    modes = _softmax(read_modes_raw, axis=-1)  # [B,R,3]

    # --- write content addressing ---
    c_w = _content_weights(memory, write_key[:, None, :], write_strength[:, None])[
        :, 0
    ]  # [B,N]

    # --- usage update + allocation ---
    retention = np.prod(
        1.0 - free[..., None] * read_w_prev, axis=1, dtype=f32
    )  # [B,N]
    usage_new = ((usage + write_w_prev - usage * write_w_prev) * retention).astype(f32)
    alloc = _allocation(usage_new)  # [B,N]

    # --- write weights, memory erase/write ---
    w_w = (g_w * (g_a * alloc + (1.0 - g_a) * c_w)).astype(f32)  # [B,N]
    memory_new = (
        memory * (1.0 - w_w[:, :, None] * erase[:, None, :])
        + w_w[:, :, None] * write_vec[:, None, :]
    ).astype(f32)  # [B,N,W]

    # --- device part: Y1 = L @ V, Y2 = L^T @ V (per batch, split in 2 slabs) ---
    # V = [rwp^T | (w*rwp)^T]  ->  [N, 8]
    V = np.concatenate(
        [
            read_w_prev.transpose(0, 2, 1),  # [B,N,R]
            (w_w[:, :, None] * read_w_prev.transpose(0, 2, 1)),
        ],
        axis=2,
    ).astype(f32)  # [B,N,8]

    # Device runs fp8 e4m3 with an exact power-of-2 prescale: values of link
    # and V are O(1/N), so x4096 recenters them into fp8's normal range. The
    # output scale (4096^2 = 2^24) divides out exactly.
    SCALE = 4096.0
    f8 = ml_dtypes.float8_e4m3
    V8 = np.clip(V * SCALE, -240.0, 240.0).astype(f8)
    link8 = (link * SCALE).astype(f8)

    in_maps = []
    for core in range(NCORES):
        b, h = divmod(core, 2)
        r0 = h * SLAB
        nat = link8[b, r0 : r0 + SLAB, :]  # [2048, 4096]
        # rows r0 + 256k + 128m + p, cols 512t + n -> [p, t, k, m, n]
        lm = np.ascontiguousarray(
            nat.reshape(8, 2, 128, NJT, 512)
            .transpose(2, 3, 0, 1, 4)
            .reshape(128, NJT * 8192)
        )
        tr = nat.T  # [4096, 2048]: [j, i-r0]
        tm = np.ascontiguousarray(
            tr.reshape(16, 2, 128, NIB, 512)
            .transpose(2, 3, 0, 1, 4)
            .reshape(128, NIB * 16384)
        )
        VS = V8[b, r0 : r0 + SLAB]  # [2048, 8]
        VF = V8[b]  # [4096, 8]
        cs = VS.reshape(8, 2, 128, 8).transpose(2, 0, 1, 3)  # [128, 8, 2, 8]
        cf = VF.reshape(16, 2, 128, 8).transpose(2, 0, 1, 3)  # [128, 16, 2, 8]
        cq = np.concatenate([cs, cf], axis=1)  # [128, 24, 2, 8]
        cpad = np.zeros((128, 24, 2, 16), dtype=f8)
        cpad[:, :, :, 0:8] = cq
        consts = np.ascontiguousarray(cpad.reshape(128, 768))
        in_maps.append({"lmat": lm, "tmat": tm, "consts": consts})

    nc = _get_nc()
    res = run_bass_kernel_spmd(
        nc,
        in_maps,
        list(range(NCORES)),
        trace=bool(os.environ.get("DNC_TRACE")),
    )
    LAST_RESULT = res

    UNSCALE = np.float32(1.0 / (SCALE * SCALE))
    Y1 = np.empty((B, N, 8), f32)
    Y2 = np.zeros((B, N, 8), f32)
    for core in range(NCORES):
        b, h = divmod(core, 2)
        r0 = h * SLAB
        Y1[b, r0 : r0 + SLAB] = res.results[core]["y1t"].T * UNSCALE
        Y2[b] += res.results[core]["y2t"].T * UNSCALE

    A = Y1[..., :R].transpose(0, 2, 1)  # [B,R,N] = (L @ rwp_r)_i
    Bm = Y1[..., R:].transpose(0, 2, 1)  # (L @ (w*rwp_r))_i
    C = Y2[..., :R].transpose(0, 2, 1)  # (L^T @ rwp_r)_i
    D = Y2[..., R:].transpose(0, 2, 1)  # (L^T @ (w*rwp_r))_i

    w = w_w[:, None, :]  # [B,1,N]
    p = precedence[:, None, :]  # [B,1,N]
    s = np.einsum("bn,brn->br", precedence, read_w_prev, dtype=f32)[..., None]
    t = np.einsum("bn,brn->br", w_w, read_w_prev, dtype=f32)[..., None]
    diag = (w * p * read_w_prev).astype(f32)  # [B,R,N]

    fwd_w = ((1.0 - w) * A - Bm + w * s - diag).astype(f32)
    bwd_w = ((1.0 - w) * C - D + p * t - diag).astype(f32)

    # --- read content addressing + combine ---
    c_r = _content_weights(memory_new, read_keys, read_strengths)  # [B,R,N]
    read_w = (
        modes[..., 0:1] * bwd_w + modes[..., 1:2] * c_r + modes[..., 2:3] * fwd_w
    ).astype(f32)
    read_vectors = np.einsum("brn,bnw->brw", read_w, memory_new, dtype=f32)
    return read_vectors.astype(f32)
